# revision 1
# baseline (speedup 1.0000x reference)
"""DeepSeek-V3 MLA attention on 8 TRN2 NeuronCores (Bass/Tile) — v2.

Same sharding as baseline (row-parallel projections, head-parallel attention)
but with per-head pipelined AllToAlls, a software-pipelined attention inner
loop (PV of tile kt-1 issues between QK of tile kt and its exp, so the PE
never waits on the softmax), in-place affine_select causal masks, and
elementwise work spread across DVE/Pool/Act.
"""
from contextlib import ExitStack

import numpy as np
import ml_dtypes

import concourse.bass as bass
import concourse.mybir as mybir
import concourse.tile as tile
from concourse import bacc
from concourse.bass_utils import run_bass_kernel_spmd

BF16NP = ml_dtypes.bfloat16
SCALE = 192 ** -0.5

dt = mybir.dt
F32, BF16 = dt.float32, dt.bfloat16

P = 128
NC_ = 8
LR = 512               # local rows per core
NH = 16
Q_LORA, KV_LORA = 1536, 512
NLAT = Q_LORA + KV_LORA + 64    # 2112
NOPE, ROPE, VH = 128, 64, 128
EPS = 1e-6
B, S = 2, 2048
R = B * S

# per-dest shard element offsets
# A2A-0a: V0 + kpe (launched right after V/kpe are ready)
VA_O = 0                  # 4 x [128, 128] rt-major
KPE_O = 128 * 512         # [64, 512]
SH0A = 192 * 512
# A2A-0b1: K0 + Qr0 (launches while Qn0 is still computing)
KN_O = 0                  # [128, 512]
QR_O = 128 * 512          # [64, 512]
SH0B1 = 192 * 512
# A2A-0b2: Qn0
SH0B2 = 128 * 512
# A2A-1: K1/Qr1/Qn1/V1 in one shot (fully hidden under attention h0)
QN1_O = 192 * 512         # [128, 512]
V_O = 320 * 512           # 4 x [128, 128] rt-major
SH1 = 448 * 512
SH2 = 128 * 512           # O send per hl


def _blk(dram, j, off, rows, width=512):
    """[rows, width] view at element offset `off` of flat shard j."""
    return dram[j, off:off + rows * width].rearrange("(p c) -> p c", c=width)


def build_kernel(reps: int = 1, debug: bool = False, loopback: bool = False):
    nc = bacc.Bacc(None, target_bir_lowering=False, debug=False)

    XT = nc.dram_tensor("xt", [2048, LR], BF16, kind="ExternalInput")
    WA = nc.dram_tensor("wa", [2048, NLAT], BF16, kind="ExternalInput")
    WQB = nc.dram_tensor("wqb", [Q_LORA, NH * 192], BF16, kind="ExternalInput")
    WKVB = nc.dram_tensor("wkvb", [KV_LORA, NH * 256], BF16, kind="ExternalInput")
    WO = nc.dram_tensor("wo", [2048, 2048], BF16, kind="ExternalInput")
    COST = nc.dram_tensor("cost", [P, LR], F32, kind="ExternalInput")
    SINT = nc.dram_tensor("sint", [P, LR], F32, kind="ExternalInput")
    MASKS = nc.dram_tensor("masks", [P, 4 * 512], BF16, kind="ExternalInput")
    OUT = nc.dram_tensor("out", [LR, 2048], F32, kind="ExternalOutput")

    SEND0A = nc.dram_tensor("send0a", [NC_, SH0A], BF16, kind="Internal")
    RECV0A = nc.dram_tensor("recv0a", [NC_, SH0A], BF16, kind="Internal")
    SEND0B1 = nc.dram_tensor("send0b1", [NC_, SH0B1], BF16, kind="Internal")
    RECV0B1 = nc.dram_tensor("recv0b1", [NC_, SH0B1], BF16, kind="Internal")
    SEND0B2 = nc.dram_tensor("send0b2", [NC_, SH0B2], BF16, kind="Internal")
    RECV0B2 = nc.dram_tensor("recv0b2", [NC_, SH0B2], BF16, kind="Internal")
    SEND1 = nc.dram_tensor("send1", [NC_, SH1], BF16, kind="Internal")
    RECV1 = nc.dram_tensor("recv1", [NC_, SH1], BF16, kind="Internal")
    SEND2 = [nc.dram_tensor(f"send2{h}", [NC_, SH2], BF16, kind="Internal")
             for h in range(2)]
    RECV2 = [nc.dram_tensor(f"recv2{h}", [NC_, SH2], BF16, kind="Internal")
             for h in range(2)]

    with tile.TileContext(nc) as tc, ExitStack() as octx:
        consts = octx.enter_context(tc.tile_pool(name="consts", bufs=1))
        ones_bf = consts.tile([P, 1], BF16)
        nc.vector.memset(ones_bf, 1.0)
        eps_t = consts.tile([1, 1], F32)
        nc.vector.memset(eps_t, EPS)
        cos_sb = consts.tile([P, LR], F32)
        sin_sb = consts.tile([P, LR], F32)
        masks = consts.tile([P, 4, 512], BF16)
        nc.gpsimd.dma_start(out=cos_sb, in_=COST[:, :])
        nc.gpsimd.dma_start(out=sin_sb, in_=SINT[:, :])
        nc.gpsimd.dma_start(
            out=masks, in_=MASKS[:, :].rearrange("p (m c) -> p m c", c=512))
        ones_row = consts.tile([1, 128], BF16)
        nc.vector.memset(ones_row, 1.0)
        cst = dict(ones_bf=ones_bf, ones_row=ones_row, cos=cos_sb,
                   sin=sin_sb, eps=eps_t, masks=masks)

        for rep in range(reps):
            _one_rep(nc, tc, rep, XT, WA, WQB, WKVB, WO, OUT,
                     SEND0A, RECV0A, SEND0B1, RECV0B1, SEND0B2, RECV0B2,
                     SEND1, RECV1, SEND2, RECV2, cst, loopback)
    nc.finalize()
    return nc


def _one_rep(nc, tc, rep, XT, WA, WQB, WKVB, WO, OUT,
             SEND0A, RECV0A, SEND0B1, RECV0B1, SEND0B2, RECV0B2,
             SEND1, RECV1, SEND2, RECV2, cst, loopback=False):
    cos_sb, sin_sb = cst["cos"], cst["sin"]
    ones_bf, eps_t = cst["ones_bf"], cst["eps"]
    ones_row, masks = cst["ones_row"], cst["masks"]

    def _a2a(send, recv):
        if loopback:
            nc.sync.dma_start(out=recv[:, :], in_=send[:, :])
        else:
            nc.gpsimd.collective_compute(
                "AllToAll", mybir.AluOpType.bypass,
                ins=[send[:, :]], outs=[recv[:, :]],
                replica_groups=[list(range(NC_))])

    with ExitStack() as ctx:
      with ExitStack() as pctx:
        s1out = pctx.enter_context(tc.tile_pool(name=f"s1out{rep}", bufs=1))
        latt = s1out.tile([P, 17, 512], BF16)     # lat^T tiles (normed in place)
        kpe_sb = s1out.tile([64, 512], BF16)      # roped k_pe^T (local rows)
        # prefetch wkvb+wqb during stage 1
        wbp = pctx.enter_context(tc.tile_pool(name=f"wb{rep}", bufs=1))
        # prefetches ride the SWDGE ring so the stage-1-critical xt/wa
        # loads own the SP HWDGE ring
        wkvb_sb = wbp.tile([P, 4, NH * 256], BF16)
        wkvb_v = WKVB[:, :].rearrange("(kt p) n -> p kt n", p=P)
        # wqb split: 9 k-tiles prefetch during stage 1, last 3 at stage-2 start
        NQA = 7
        wqb_a = wbp.tile([P, NQA, NH * 192], BF16)
        wqb_v = WQB[:, :].rearrange("(kt p) n -> p kt n", p=P)
        # issued below, behind xt/wa/wkvb on the SP FIFO

        # ---------------- Stage 1: lat^T = WA^T @ x^T + rmsnorm ------------
        with ExitStack() as sctx:
            wap = sctx.enter_context(tc.tile_pool(name=f"wa{rep}", bufs=1))
            pp1 = sctx.enter_context(tc.tile_pool(name=f"ps1{rep}", bufs=4, space="PSUM"))
            ppq = sctx.enter_context(tc.tile_pool(name=f"psq{rep}", bufs=2, space="PSUM"))
            sqp = sctx.enter_context(tc.tile_pool(name=f"sq{rep}", bufs=3))
            nrm = sctx.enter_context(tc.tile_pool(name=f"nrm{rep}", bufs=1))

            wa_sb = wap.tile([P, 16, NLAT], BF16)
            xt_sb = wap.tile([P, 16, LR], BF16)
            wa_v = WA[:, :].rearrange("(kt p) n -> p kt n", p=P)
            xt_v = XT[:, :].rearrange("(kt p) n -> p kt n", p=P)
            # first tiles land as singles so the first MM starts in ~2us;
            # the rest pair up to halve HWDGE descriptor slots
            for kt in (0, 1):
                nc.sync.dma_start(out=xt_sb[:, kt:kt + 1, :],
                                  in_=xt_v[:, kt:kt + 1, :])
                nc.sync.dma_start(out=wa_sb[:, kt:kt + 1, :],
                                  in_=wa_v[:, kt:kt + 1, :])
            for kt in range(2, 16, 2):
                nc.sync.dma_start(out=xt_sb[:, kt:kt + 2, :],
                                  in_=xt_v[:, kt:kt + 2, :])
                nc.sync.dma_start(out=wa_sb[:, kt:kt + 2, :],
                                  in_=wa_v[:, kt:kt + 2, :])
            # wkvb behind xt/wa on the same SP FIFO: starts only once the
            # stage-1-critical loads are done, ready in time for stage-2 V
            nc.sync.dma_start(out=wkvb_sb[:, :, :], in_=wkvb_v[:, 0:4, :])
            nc.sync.dma_start(out=wqb_a[:, :, :], in_=wqb_v[:, 0:NQA, :])

            ps_ssq_q = ppq.tile([1, 512], F32)
            ps_ssq_kv = ppq.tile([1, 512], F32)
            rq = nrm.tile([1, 512], F32)
            rkv = nrm.tile([1, 512], F32)
            rq_b = nrm.tile([P, 512], F32)
            rkv_b = nrm.tile([P, 512], F32)
            for pt in range(17):
                pw = 128 if pt < 16 else 64
                ps = pp1.tile([P, 512], F32)
                for kt in range(16):
                    nc.tensor.matmul(
                        ps[:pw, :], lhsT=wa_sb[:, kt, pt * 128:pt * 128 + pw],
                        rhs=xt_sb[:, kt, :], start=(kt == 0), stop=(kt == 15))
                nc.scalar.copy(latt[:pw, pt, :], ps[:pw, :])
                if pt < 16:
                    # streamed sum-of-squares: square into a rotating tile,
                    # immediately fold into the running PE reduction
                    sq_t = sqp.tile([P, 512], BF16, tag="sq")
                    nc.scalar.square(sq_t, ps)
                    if pt < 12:
                        nc.tensor.matmul(ps_ssq_q, lhsT=ones_bf, rhs=sq_t,
                                         start=(pt == 0), stop=(pt == 11))
                    else:
                        nc.tensor.matmul(ps_ssq_kv, lhsT=ones_bf, rhs=sq_t,
                                         start=(pt == 12), stop=(pt == 15))
                if pt == 11:
                    nc.scalar.activation(rq, ps_ssq_q,
                                         mybir.ActivationFunctionType.Sqrt,
                                         bias=eps_t, scale=1.0 / Q_LORA)
                    nc.vector.reciprocal(rq, rq)
                    nc.gpsimd.partition_broadcast(rq_b, rq)
                if pt == 15:
                    nc.scalar.activation(rkv, ps_ssq_kv,
                                         mybir.ActivationFunctionType.Sqrt,
                                         bias=eps_t, scale=1.0 / KV_LORA)
                    nc.vector.reciprocal(rkv, rkv)
                    nc.gpsimd.partition_broadcast(rkv_b, rkv)
            for pt in range(12, 16):
                eng = nc.vector if pt % 2 == 0 else nc.gpsimd
                eng.tensor_mul(latt[:, pt, :], latt[:, pt, :], rkv_b)
            for pt in range(12):
                eng = nc.vector if pt % 2 == 0 else nc.gpsimd
                eng.tensor_mul(latt[:, pt, :], latt[:, pt, :], rq_b)

            # k_pe rope (unnormed): latt[:, 16, :] rows [e(32)|o(32)]
            kp = nrm.tile([32, 4, 512], BF16, tag="krope")
            xo_c = nrm.tile([32, 512], BF16, tag="kxo")
            nc.sync.dma_start(out=xo_c, in_=latt[32:64, 16, :])
            xe = latt[0:32, 16, :]
            c32, s32 = cos_sb[0:32, :], sin_sb[0:32, :]
            nc.vector.tensor_mul(kp[:, 0, :], xe, c32)
            nc.vector.tensor_mul(kp[:, 1, :], xe, s32)
            nc.vector.tensor_mul(kp[:, 2, :], xo_c, s32)
            nc.vector.tensor_mul(kp[:, 3, :], xo_c, c32)
            nc.vector.tensor_sub(kpe_sb[0:32, :], kp[:, 0, :], kp[:, 2, :])
            yi = nrm.tile([32, 512], BF16, tag="kyi")
            nc.vector.tensor_add(yi, kp[:, 1, :], kp[:, 3, :])
            nc.sync.dma_start(out=kpe_sb[32:64, :], in_=yi)

        # ---------------- Stage 2: V/K/Q with per-head-parity A2As ---------
        s2out = pctx.enter_context(tc.tile_pool(name=f"s2out{rep}", bufs=1))
        with ExitStack() as sctx:
            pp2 = sctx.enter_context(tc.tile_pool(name=f"ps2{rep}", bufs=4, space="PSUM"))
            rp = sctx.enter_context(tc.tile_pool(name=f"qrope{rep}", bufs=2))
            wqb_b = s2out.tile([P, 12 - NQA, NH * 192], BF16)
            for kt in range(NQA, 12):
                nc.scalar.dma_start(out=wqb_b[:, kt - NQA, :], in_=wqb_v[:, kt, :])

            def wqb_t(kt):
                return wqb_a[:, kt, :] if kt < NQA else wqb_b[:, kt - NQA, :]

            # v_sb cols = par*1024 + (h//2)*128 + vh  (parity-major)
            v_sb = s2out.tile([P, 4, 2048], BF16)
            kt_sb = s2out.tile([P, 16, 512], BF16)
            qt_sb = s2out.tile([P, 24, 512], BF16)
            wkvb_p = wkvb_sb.rearrange(
                "p kt (hp par two vh) -> p kt par hp two vh",
                par=2, two=2, vh=128)

            def v_par(par):
                for gi in range(2):
                    for rt in range(4):
                        ps = pp2.tile([P, 512], F32)
                        rhs = wkvb_p[:, :, par, 4 * gi:4 * gi + 4, 1, :]
                        for kt in range(4):
                            nc.tensor.matmul(
                                ps, lhsT=latt[:, 12 + kt, rt * 128:(rt + 1) * 128],
                                rhs=rhs[:, kt, :, :], start=(kt == 0), stop=(kt == 3))
                        nc.vector.tensor_copy(
                            v_sb[:, rt, par * 1024 + gi * 512:par * 1024 + (gi + 1) * 512], ps)

            def k_par(par):
                for h in range(par, NH, 2):
                    ps = pp2.tile([P, 512], F32)
                    for kt in range(4):
                        nc.tensor.matmul(
                            ps, lhsT=wkvb_sb[:, kt, h * 256:h * 256 + 128],
                            rhs=latt[:, 12 + kt, :], start=(kt == 0), stop=(kt == 3))
                    nc.vector.tensor_copy(kt_sb[:, h, :], ps)

            def q_tiles(tiles):
                for pt in tiles:
                    ps = pp2.tile([P, 512], F32)
                    for kt in range(12):
                        nc.tensor.matmul(
                            ps, lhsT=wqb_t(kt)[:, pt * 128:(pt + 1) * 128],
                            rhs=latt[:, kt, :], start=(kt == 0), stop=(kt == 11))
                    nc.scalar.copy(qt_sb[:, pt, :], ps)

            def send_v(par, SEND, voff):
                # one 3D DMA per dest on the Act HWDGE ring
                for j in range(NC_):
                    nc.scalar.dma_start(
                        out=SEND[j, voff:voff + 4 * 128 * 128].rearrange(
                            "(rt p c) -> p rt c", rt=4, p=128),
                        in_=v_sb[:, :, par * 1024 + j * 128:par * 1024 + (j + 1) * 128])

            def send_kqr(par, SEND):
                for j in range(NC_):
                    h = 2 * j + par
                    nc.scalar.dma_start(out=_blk(SEND, j, KN_O, 128),
                                        in_=kt_sb[:, h, :])
                    pe = (h % 4) * 32
                    nc.scalar.dma_start(out=_blk(SEND, j, QR_O, 32),
                                        in_=qt_sb[pe:pe + 32, 16 + h // 4, :])
                    nc.scalar.dma_start(out=_blk(SEND, j, QR_O + 32 * 512, 32),
                                        in_=qt_sb[pe:pe + 32, 20 + h // 4, :])

            def send_qn(par, SEND, off):
                for j in range(NC_):
                    nc.scalar.dma_start(out=_blk(SEND, j, off, 128),
                                        in_=qt_sb[:, 2 * j + par, :])

            # A2A-0a: V of even heads + kpe, launched as early as possible
            v_par(0)
            for j in range(NC_):
                nc.scalar.dma_start(out=_blk(SEND0A, j, KPE_O, 64), in_=kpe_sb)
            send_v(0, SEND0A, VA_O)
            _a2a(SEND0A, RECV0A)

            # A2A-0b1: K/Qr of even heads (in flight during Qn0 compute)
            k_par(0)
            q_tiles(range(16, 24))      # all rope tiles
            # Q rope in place (bf16 temps; e-tiles 16+j vs o-tiles 20+j)
            for j in range(4):
                et = qt_sb[:, 16 + j, :]
                ot = qt_sb[:, 20 + j, :]
                t = rp.tile([P, 4, 512], BF16, tag="qr")
                nc.vector.tensor_mul(t[:, 0, :], et, cos_sb)
                nc.vector.tensor_mul(t[:, 1, :], et, sin_sb)
                nc.vector.tensor_mul(t[:, 2, :], ot, sin_sb)
                nc.vector.tensor_mul(t[:, 3, :], ot, cos_sb)
                nc.vector.tensor_sub(et, t[:, 0, :], t[:, 2, :])
                nc.vector.tensor_add(ot, t[:, 1, :], t[:, 3, :])
            send_kqr(0, SEND0B1)
            _a2a(SEND0B1, RECV0B1)

            # A2A-0b2: Qn of even heads; each dest's send fires as soon as
            # its tile is copied so the collective triggers with no send lag
            for j in range(NC_):
                q_tiles([2 * j])
                nc.scalar.dma_start(out=_blk(SEND0B2, j, 0, 128),
                                    in_=qt_sb[:, 2 * j, :])
            _a2a(SEND0B2, RECV0B2)

            # A2A-1: everything for odd heads
            v_par(1)
            k_par(1)
            q_tiles(range(1, 16, 2))
            send_kqr(1, SEND1)
            send_qn(1, SEND1, QN1_O)
            send_v(1, SEND1, V_O)
            _a2a(SEND1, RECV1)
      # projection pools (latt/wkvb/wqb/qt/kt/v) freed here
      if True:
        # ---------------- Stage 4: attention (hl outer, b inner) -----------
        wop = ctx.enter_context(tc.tile_pool(name=f"wo{rep}", bufs=1))
        wo_sb = wop.tile([P, 16, 2048], BF16)
        wo_v = WO[:, :].rearrange("(kt p) n -> p kt n", p=P)
        for kt in range(16):
            # SWDGE ring: bulk prefetch that must not block the attention
            # recv DMAs on the HWDGE rings
            nc.gpsimd.dma_start(out=wo_sb[:, kt, :], in_=wo_v[:, kt, :])
        otf = wop.tile([P, 16, 512], BF16)

        with ExitStack() as sctx:
            asm = sctx.enter_context(tc.tile_pool(name=f"asm{rep}", bufs=3))
            ptp = sctx.enter_context(tc.tile_pool(name=f"pt{rep}", bufs=6))
            ppS = sctx.enter_context(tc.tile_pool(name=f"psS{rep}", bufs=3, space="PSUM"))
            ppO = sctx.enter_context(tc.tile_pool(name=f"psO{rep}", bufs=4, space="PSUM"))
            ppD = sctx.enter_context(tc.tile_pool(name=f"psD{rep}", bufs=1, space="PSUM"))
            sml = sctx.enter_context(tc.tile_pool(name=f"sml{rep}", bufs=3))
            otp = sctx.enter_context(tc.tile_pool(name=f"ot{rep}", bufs=2))

            kpool = sctx.enter_context(tc.tile_pool(name=f"kpe{rep}", bufs=1))
            kpe_all = kpool.tile([64, 8, 512], BF16)
            nc.sync.dma_start(
                out=kpe_all,
                in_=RECV0A[:, KPE_O:KPE_O + 64 * 512].rearrange(
                    "s (p c) -> p s c", c=512))

            for hl in range(2):
                RECV = RECV0B1 if hl == 0 else RECV1
                RECVQ = RECV0B2 if hl == 0 else RECV1
                qoff = 0 if hl == 0 else QN1_O
                RECVV = RECV0A if hl == 0 else RECV1
                voff = VA_O if hl == 0 else V_O
                ot_sb = otp.tile([P, 4096], BF16, tag="ot")
                for b in range(B):
                    ktn = asm.tile([P, 4, 512], BF16, tag="ktn")
                    qtn = asm.tile([P, 4, 512], BF16, tag="qtn")
                    qtr = asm.tile([64, 4, 512], BF16, tag="qtr")
                    vt = asm.tile([P, 16, 128], BF16, tag="vt")
                    s0 = 4 * b
                    nc.sync.dma_start(
                        out=ktn,
                        in_=RECV[s0:s0 + 4, KN_O:KN_O + 128 * 512].rearrange(
                            "s (p c) -> p s c", c=512))
                    nc.sync.dma_start(
                        out=qtr,
                        in_=RECV[s0:s0 + 4, QR_O:QR_O + 64 * 512].rearrange(
                            "s (p c) -> p s c", c=512))
                    nc.sync.dma_start(
                        out=qtn,
                        in_=RECVQ[s0:s0 + 4, qoff:qoff + 128 * 512].rearrange(
                            "s (p c) -> p s c", c=512))
                    for i in range(4):
                        nc.sync.dma_start(
                            out=vt[:, 4 * i:4 * i + 4, :],
                            in_=RECVV[s0 + i, voff:voff + 4 * 128 * 128].rearrange(
                                "(rt p c) -> p rt c", rt=4, c=128))
                    for qg in range(4):
                        psO = ppO.tile([P, 512], F32)
                        nkt = 4 * qg + 4
                        dacc_a = sml.tile([P, 512], BF16, tag="dacca")
                        prev = None
                        for kt in range(nkt):
                            m = kt - 4 * qg
                            # diagonal tile m only has live queries in columns
                            # [128m, 512) — skip the dead left part. kt<2 stay
                            # full so dacc copies / psO group init every col.
                            lo = 128 * m if (m > 0 and kt >= 2) else 0
                            cs = slice(lo, 512)
                            psS = ppS.tile([P, 512], F32)
                            nc.tensor.matmul(
                                psS[:, cs],
                                lhsT=ktn[:, kt // 4, (kt % 4) * 128:(kt % 4 + 1) * 128],
                                rhs=qtn[:, qg, cs], start=True, stop=False)
                            nc.tensor.matmul(
                                psS[:, cs],
                                lhsT=kpe_all[:, 4 * b + kt // 4, (kt % 4) * 128:(kt % 4 + 1) * 128],
                                rhs=qtr[:, qg, cs], start=False, stop=True)
                            if prev is not None:
                                ppt, pcs = prev
                                nc.tensor.matmul(psO[:, pcs], lhsT=vt[:, kt - 1, :],
                                                 rhs=ppt[:, pcs],
                                                 start=(kt == 1), stop=False)
                            pt_t = ptp.tile([P, 512], BF16, tag="pt")
                            nc.scalar.activation(pt_t[:, cs], psS[:, cs],
                                                 mybir.ActivationFunctionType.Exp)
                            if m >= 0:
                                nc.vector.tensor_mul(pt_t[:, cs], pt_t[:, cs],
                                                     masks[:, m, cs])
                            if kt == 0:
                                nc.vector.tensor_copy(dacc_a, pt_t)
                            else:
                                nc.vector.tensor_add(dacc_a[:, cs], dacc_a[:, cs],
                                                     pt_t[:, cs])
                            prev = (pt_t, cs)
                        ppt, pcs = prev
                        nc.tensor.matmul(psO[:, pcs], lhsT=vt[:, nkt - 1, :],
                                         rhs=ppt[:, pcs],
                                         start=(nkt == 1), stop=True)
                        psD = ppD.tile([1, 512], F32)
                        nc.tensor.matmul(psD, lhsT=ones_bf, rhs=dacc_a,
                                         start=True, stop=True)
                        rcp = sml.tile([1, 512], F32, tag="rcp")
                        nc.vector.reciprocal(rcp, psD)
                        rdb = sml.tile([P, 512], F32, tag="rdb")
                        nc.gpsimd.partition_broadcast(rdb, rcp)
                        nc.vector.tensor_mul(
                            ot_sb[:, b * 2048 + qg * 512:b * 2048 + (qg + 1) * 512],
                            psO, rdb)
                # ship this head, overlap with next head's attention
                nc.scalar.dma_start(
                    out=SEND2[hl][:, :].rearrange("j (p c) -> p j c", c=512),
                    in_=ot_sb[:, :].rearrange("p (j c) -> p j c", c=512))
                _a2a(SEND2[hl], RECV2[hl])
                nc.sync.dma_start(
                    out=otf[:, hl:16:2, :],
                    in_=RECV2[hl][:, :].rearrange("j (p c) -> p j c", c=512))

        # ---------------- Stage 6: out = O^T.T @ WO, hl-split --------------
        # pass 1 (even heads, from the h0 A2A) runs while the h1 O-A2A is in
        # flight; pass 2 adds the odd heads and streams the output out.
        with ExitStack() as sctx:
            pp6 = sctx.enter_context(tc.tile_pool(name=f"ps6{rep}", bufs=4, space="PSUM"))
            outp = sctx.enter_context(tc.tile_pool(name=f"outp{rep}", bufs=2))
            acc = outp.tile([P, 4, 2048], F32, tag="acc")
            for rt in range(4):
                for ng in range(4):
                    ps = pp6.tile([P, 512], F32)
                    for i, h in enumerate(range(0, 16, 2)):
                        nc.tensor.matmul(
                            ps, lhsT=otf[:, h, rt * 128:(rt + 1) * 128],
                            rhs=wo_sb[:, h, ng * 512:(ng + 1) * 512],
                            start=(i == 0), stop=(i == 7))
                    nc.scalar.copy(acc[:, rt, ng * 512:(ng + 1) * 512], ps)
            for rt in range(4):
                out_t = outp.tile([P, 2048], F32, tag="outt")
                for ng in range(4):
                    ps = pp6.tile([P, 512], F32)
                    for i, h in enumerate(range(1, 16, 2)):
                        nc.tensor.matmul(
                            ps, lhsT=otf[:, h, rt * 128:(rt + 1) * 128],
                            rhs=wo_sb[:, h, ng * 512:(ng + 1) * 512],
                            start=(i == 0), stop=(i == 7))
                    nc.vector.tensor_add(
                        out_t[:, ng * 512:(ng + 1) * 512],
                        acc[:, rt, ng * 512:(ng + 1) * 512], ps)
                    nc.sync.dma_start(
                        out=OUT[rt * 128:(rt + 1) * 128, ng * 512:(ng + 1) * 512],
                        in_=out_t[:, ng * 512:(ng + 1) * 512])


# ---------------------------------------------------------------------------
# Host-side prep (identical to baseline)
# ---------------------------------------------------------------------------

def _bf(a):
    return np.asarray(a, dtype=np.float32).astype(BF16NP)


def _prep_weights(wq_a, q_norm_w, wq_b, wkv_a, kv_norm_w, wkv_b, wo,
                  freqs_cos, freqs_sin):
    wkv_a_lat = wkv_a[:, :KV_LORA]
    wkv_a_rope = wkv_a[:, KV_LORA:]
    wkv_a_rope = np.concatenate([wkv_a_rope[:, 0::2], wkv_a_rope[:, 1::2]], axis=1)
    WAh = np.concatenate([wq_a, wkv_a_lat, wkv_a_rope], axis=1)      # [2048, 2112]

    wqb = (wq_b * SCALE) * q_norm_w[:, None]
    wqb = wqb.reshape(Q_LORA, NH, 192)
    nope_cols = wqb[:, :, :NOPE].reshape(Q_LORA, NH * NOPE)
    rope_e = wqb[:, :, NOPE + 0::2].reshape(Q_LORA, NH * 32)
    rope_o = wqb[:, :, NOPE + 1::2].reshape(Q_LORA, NH * 32)
    WQBh = np.concatenate([nope_cols, rope_e, rope_o], axis=1)       # [1536, 3072]

    WKVBh = wkv_b * kv_norm_w[:, None]                                # [512, 4096]
    pos = np.arange(R) % S
    COS = freqs_cos[pos].astype(np.float32)                           # [4096, 32]
    SIN = freqs_sin[pos].astype(np.float32)
    p = np.arange(128)[:, None]
    c = np.arange(512)[None, :]
    MASK = np.stack([(c - 128 * m - p >= 0) for m in range(4)],
                    axis=1).astype(np.float32)                        # [128, 4, 512]
    return dict(WA=_bf(WAh), WQB=_bf(WQBh), WKVB=_bf(WKVBh), WO=_bf(wo),
                COS=COS, SIN=SIN, MASK=_bf(MASK.reshape(128, 2048)))


def _prep_in_maps(inputs):
    x = np.asarray(inputs["x"], dtype=np.float32).reshape(R, 2048)
    W = _prep_weights(
        np.asarray(inputs["wq_a"]), np.asarray(inputs["q_norm_w"]),
        np.asarray(inputs["wq_b"]), np.asarray(inputs["wkv_a"]),
        np.asarray(inputs["kv_norm_w"]), np.asarray(inputs["wkv_b"]),
        np.asarray(inputs["wo"]),
        np.asarray(inputs["freqs_cos"]), np.asarray(inputs["freqs_sin"]))
    in_maps = []
    for c in range(NC_):
        rows = slice(c * LR, (c + 1) * LR)
        in_maps.append({
            "xt": np.ascontiguousarray(x[rows].T).astype(BF16NP),
            "wa": W["WA"], "wqb": W["WQB"], "wkvb": W["WKVB"], "wo": W["WO"],
            "cost": np.ascontiguousarray(np.tile(W["COS"][rows].T, (4, 1))),
            "sint": np.ascontiguousarray(np.tile(W["SIN"][rows].T, (4, 1))),
            "masks": W["MASK"],
        })
    return in_maps


prep_in_maps = _prep_in_maps

_NC_CACHE = []


def _get_nc():
    if not _NC_CACHE:
        _NC_CACHE.append(build_kernel())
    return _NC_CACHE[0]


def kernel(**inputs) -> np.ndarray:
    in_maps = _prep_in_maps(inputs)
    nc = _get_nc()
    res = run_bass_kernel_spmd(nc, in_maps, core_ids=list(range(NC_)))
    outs = [res.results[c]["out"] for c in range(NC_)]
    return np.concatenate(outs, axis=0).reshape(B, S, 2048).astype(np.float32)



# revision 27
# speedup vs baseline: 1.4508x; 1.4508x over previous
"""DeepSeek-V3 MLA attention on 8 TRN2 NeuronCores (Bass/Tile) — v3.

Same sharding as v2 (row-parallel projections, head-parallel attention),
plus:
- rmsnorm folded into the stage-2 PSUM->SBUF copies (the norm scale is
  per-token = per output column), so no normalized-latent pass exists and
  stage-2 matmuls never wait on the sum-of-squares reduction.
- stage 1 runs kt-outer over pt-chunks of 4 PSUM banks, kv-latent columns
  first, so the PE rides the wa/xt DMA stream instead of stalling on the
  full 10.8 MB load, and V-projection matmuls interleave into the stage-1
  instruction stream right after the kv chunk completes.
- A2A-0a (V even + kpe) therefore fires mid-stage-1.
- softmax 1/sum chain is deferred and emitted two k-tiles into the next
  query group, removing PE head-of-line blocking on the DVE dacc chain.
- attention inputs for all (head, batch) pairs are DMA'd up front.
"""
from collections import deque
from contextlib import ExitStack

import numpy as np
import ml_dtypes

import concourse.bass as bass
import concourse.mybir as mybir
import concourse.tile as tile
from concourse import bacc
from concourse.bass_utils import run_bass_kernel_spmd

BF16NP = ml_dtypes.bfloat16
SCALE = 192 ** -0.5

dt = mybir.dt
F32, BF16 = dt.float32, dt.bfloat16

P = 128
NC_ = 8
LR = 512               # local rows per core
NH = 16
Q_LORA, KV_LORA = 1536, 512
NLAT = Q_LORA + KV_LORA + 64    # 2112
NOPE, ROPE, VH = 128, 64, 128
EPS = 1e-6
B, S = 2, 2048
R = B * S

# per-dest shard element offsets (unchanged from v2)
VA_O = 0                  # 4 x [128, 128] rt-major
KPE_O = 128 * 512         # [64, 512]
SH0A = 192 * 512
KN_O = 0                  # [128, 512]
QR_O = 128 * 512          # [64, 512]
SH0B1 = 192 * 512
SH0B2 = 128 * 512
QN1_O = 192 * 512         # [128, 512]
V_O = 320 * 512           # 4 x [128, 128] rt-major
SH1 = 448 * 512
SH2 = 128 * 512           # O send per hl

# stage-1 pt chunks (kv-latent first so V can start early); pt16 = k_pe rows
CHUNKS = [[12, 13, 14, 15], [16, 0, 1, 2], [3, 4, 5, 6], [7, 8, 9, 10], [11]]

# wqb host column layout: [rope_e 512 | rope_o 512 | even nope 1024 | odd 1024]
def _qcol(pt):
    if 16 <= pt < 20:
        return (pt - 16) * 128
    if 20 <= pt < 24:
        return 512 + (pt - 20) * 128
    if pt % 2 == 0:
        return 1024 + (pt // 2) * 128
    return 2048 + (pt // 2) * 128


def _blk(dram, j, off, rows, width=512):
    """[rows, width] view at element offset `off` of flat shard j."""
    return dram[j, off:off + rows * width].rearrange("(p c) -> p c", c=width)


def build_kernel(reps: int = 1, debug: bool = False, loopback: bool = False):
    nc = bacc.Bacc(None, target_bir_lowering=False, debug=False)

    XT = nc.dram_tensor("xt", [2048, LR], BF16, kind="ExternalInput")
    WA = nc.dram_tensor("wa", [2048, NLAT], BF16, kind="ExternalInput")
    WQB = nc.dram_tensor("wqb", [Q_LORA, NH * 192], BF16, kind="ExternalInput")
    WKVB = nc.dram_tensor("wkvb", [KV_LORA, NH * 256], BF16, kind="ExternalInput")
    WO = nc.dram_tensor("wo", [2048, 2048], BF16, kind="ExternalInput")
    COST = nc.dram_tensor("cost", [P, LR], F32, kind="ExternalInput")
    SINT = nc.dram_tensor("sint", [P, LR], F32, kind="ExternalInput")
    MASKS = nc.dram_tensor("masks", [P, 4 * 512], BF16, kind="ExternalInput")
    OUT = nc.dram_tensor("out", [LR, 2048], F32, kind="ExternalOutput")

    SEND0A = nc.dram_tensor("send0a", [NC_, SH0A], BF16, kind="Internal")
    RECV0A = nc.dram_tensor("recv0a", [NC_, SH0A], BF16, kind="Internal")
    SEND0B1 = nc.dram_tensor("send0b1", [NC_, SH0B1], BF16, kind="Internal")
    RECV0B1 = nc.dram_tensor("recv0b1", [NC_, SH0B1], BF16, kind="Internal")
    SEND0B2 = nc.dram_tensor("send0b2", [NC_, SH0B2], BF16, kind="Internal")
    RECV0B2 = nc.dram_tensor("recv0b2", [NC_, SH0B2], BF16, kind="Internal")
    SEND1 = nc.dram_tensor("send1", [NC_, SH1], BF16, kind="Internal")
    RECV1 = nc.dram_tensor("recv1", [NC_, SH1], BF16, kind="Internal")
    SEND2 = [nc.dram_tensor(f"send2{h}", [NC_, SH2], BF16, kind="Internal")
             for h in range(2)]
    RECV2 = [nc.dram_tensor(f"recv2{h}", [NC_, SH2], BF16, kind="Internal")
             for h in range(2)]
    RKVT = nc.dram_tensor("rkvt", [LR], F32, kind="Internal")

    with tile.TileContext(nc) as tc, ExitStack() as octx:
        consts = octx.enter_context(tc.tile_pool(name="consts", bufs=1))
        ones_bf = consts.tile([P, 1], BF16)
        nc.vector.memset(ones_bf, 1.0)
        eps_t = consts.tile([1, 1], F32)
        nc.vector.memset(eps_t, EPS)
        cos_sb = consts.tile([P, LR], F32)
        sin_sb = consts.tile([P, LR], F32)
        masks = consts.tile([P, 4, 512], BF16)
        nc.gpsimd.dma_start(out=cos_sb, in_=COST[:, :])
        nc.gpsimd.dma_start(out=sin_sb, in_=SINT[:, :])
        nc.gpsimd.dma_start(
            out=masks, in_=MASKS[:, :].rearrange("p (m c) -> p m c", c=512))
        ones_row = consts.tile([1, 128], BF16)
        nc.vector.memset(ones_row, 1.0)
        cst = dict(ones_bf=ones_bf, ones_row=ones_row, cos=cos_sb,
                   sin=sin_sb, eps=eps_t, masks=masks)

        for rep in range(reps):
            _one_rep(nc, tc, rep, XT, WA, WQB, WKVB, WO, OUT,
                     SEND0A, RECV0A, SEND0B1, RECV0B1, SEND0B2, RECV0B2,
                     SEND1, RECV1, SEND2, RECV2, RKVT, cst, loopback)
    nc.finalize()
    return nc


def _one_rep(nc, tc, rep, XT, WA, WQB, WKVB, WO, OUT,
             SEND0A, RECV0A, SEND0B1, RECV0B1, SEND0B2, RECV0B2,
             SEND1, RECV1, SEND2, RECV2, RKVT, cst, loopback=False):
    cos_sb, sin_sb = cst["cos"], cst["sin"]
    ones_bf, eps_t = cst["ones_bf"], cst["eps"]
    masks = cst["masks"]

    def _a2a(send, recv):
        if loopback:
            nc.sync.dma_start(out=recv[:, :], in_=send[:, :])
        else:
            nc.gpsimd.collective_compute(
                "AllToAll", mybir.AluOpType.bypass,
                ins=[send[:, :]], outs=[recv[:, :]],
                replica_groups=[list(range(NC_))])

    with ExitStack() as ctx:
      with ExitStack() as pctx:
        s1out = pctx.enter_context(tc.tile_pool(name=f"s1out{rep}", bufs=1))
        latt = s1out.tile([P, 17, 512], BF16)     # unnormed lat^T tiles
        kpe_sb = s1out.tile([64, 512], BF16)      # roped k_pe^T (local rows)
        # rmsnorm reciprocal scales (per token = per column), built during
        # stage 1, consumed by stage-2 psum->sbuf copies
        rq = s1out.tile([1, 512], F32)
        rkv = s1out.tile([1, 512], F32)
        rq_b = s1out.tile([P, 512], F32)
        rkv_b = s1out.tile([P, 512], F32)
        # rkv transposed to [token%128, token//128]: V's PSUM rows are tokens,
        # so its norm scale is per-partition, not per-column
        rkv_t = s1out.tile([P, 4], F32)
        # v_sb cols = par*1024 + (h//2)*128 + vh  (parity-major)
        v_sb = s1out.tile([P, 4, 2048], BF16)
        # prefetch wkvb+wqb during stage 1 behind wa/xt on the SP FIFO
        wbp = pctx.enter_context(tc.tile_pool(name=f"wb{rep}", bufs=1))
        wkvb_sb = wbp.tile([P, 4, NH * 256], BF16)
        wkvb_v = WKVB[:, :].rearrange("(kt p) n -> p kt n", p=P)
        NQA = 4
        wqb_a = wbp.tile([P, NQA, NH * 192], BF16)
        wqb_v = WQB[:, :].rearrange("(kt p) n -> p kt n", p=P)
        # 2 PSUM banks for the V matmuls that interleave into stage 1
        pp2 = pctx.enter_context(tc.tile_pool(name=f"ps2{rep}", bufs=2, space="PSUM"))

        wkvb_p = wkvb_sb.rearrange(
            "p kt (hp par two vh) -> p kt par hp two vh",
            par=2, two=2, vh=128)

        def v_group(par, gi, rt):
            ps = pp2.tile([P, 512], F32, tag="vps")
            rhs = wkvb_p[:, :, par, 4 * gi:4 * gi + 4, 1, :]
            for kt in range(4):
                nc.tensor.matmul(
                    ps, lhsT=latt[:, 12 + kt, rt * 128:(rt + 1) * 128],
                    rhs=rhs[:, kt, :, :], start=(kt == 0), stop=(kt == 3))
            nc.scalar.activation(
                v_sb[:, rt, par * 1024 + gi * 512:par * 1024 + (gi + 1) * 512],
                ps, mybir.ActivationFunctionType.Copy,
                scale=rkv_t[:, rt:rt + 1])

        # ---------------- Stage 1 + early V: kt-outer pt-chunks ------------
        with ExitStack() as sctx:
            wap = sctx.enter_context(tc.tile_pool(name=f"wa{rep}", bufs=1))
            pp1 = sctx.enter_context(tc.tile_pool(name=f"ps1{rep}", bufs=4, space="PSUM"))
            ppq = sctx.enter_context(tc.tile_pool(name=f"psq{rep}", bufs=1, space="PSUM"))
            sqp = sctx.enter_context(tc.tile_pool(name=f"sq{rep}", bufs=4))
            nrm = sctx.enter_context(tc.tile_pool(name=f"nrm{rep}", bufs=1))

            wa_sb = wap.tile([P, 16, NLAT], BF16)
            xt_sb = wap.tile([P, 16, LR], BF16)
            wa_v = WA[:, :].rearrange("(kt p) n -> p kt n", p=P)
            xt_v = XT[:, :].rearrange("(kt p) n -> p kt n", p=P)
            # kv-latent (+k_pe) columns stream first, kt-interleaved with xt,
            # so the first chunk's matmuls ride the DMA arrivals
            for kt in range(16):
                nc.sync.dma_start(out=xt_sb[:, kt:kt + 1, :],
                                  in_=xt_v[:, kt:kt + 1, :])
                nc.sync.dma_start(out=wa_sb[:, kt, 1536:2112],
                                  in_=wa_v[:, kt, 1536:2112])
            for kt in range(0, 16, 2):
                nc.sync.dma_start(out=wa_sb[:, kt:kt + 2, 0:384],
                                  in_=wa_v[:, kt:kt + 2, 0:384])
            # wkvb mid-stream: in SBUF before the V phase after chunk 3
            nc.sync.dma_start(out=wkvb_sb[:, :, :], in_=wkvb_v[:, 0:4, :])
            for c0, c1 in ((384, 896), (896, 1408), (1408, 1536)):
                for kt in range(0, 16, 2):
                    nc.sync.dma_start(out=wa_sb[:, kt:kt + 2, c0:c1],
                                      in_=wa_v[:, kt:kt + 2, c0:c1])
            nc.sync.dma_start(out=wqb_a[:, :, :], in_=wqb_v[:, 0:NQA, :])

            ps_ssq_q = ppq.tile([1, 512], F32)
            ps_ssq_kv = ppq.tile([1, 512], F32)

            def ssq_mm(pt):
                def emit(sq_t):
                    if pt < 12:
                        nc.tensor.matmul(ps_ssq_q, lhsT=ones_bf, rhs=sq_t,
                                         start=(pt == 0), stop=(pt == 11))
                    else:
                        nc.tensor.matmul(ps_ssq_kv, lhsT=ones_bf, rhs=sq_t,
                                         start=(pt == 12), stop=(pt == 15))
                return emit

            def rkv_chain():
                nc.scalar.activation(rkv, ps_ssq_kv,
                                     mybir.ActivationFunctionType.Sqrt,
                                     bias=eps_t, scale=1.0 / KV_LORA)
                nc.vector.reciprocal(rkv, rkv)
                nc.gpsimd.partition_broadcast(rkv_b, rkv)
                # single-partition -> 128-partition scatter must roundtrip
                # through DRAM (SBUF->SBUF partition scatter silently fails)
                nc.scalar.dma_start(out=RKVT[:], in_=rkv[0:1, :])
                nc.scalar.dma_start(
                    out=rkv_t, in_=RKVT[:].rearrange("(rt p) -> p rt", p=128))

            def rq_chain():
                nc.scalar.activation(rq, ps_ssq_q,
                                     mybir.ActivationFunctionType.Sqrt,
                                     bias=eps_t, scale=1.0 / Q_LORA)
                nc.vector.reciprocal(rq, rq)
                nc.gpsimd.partition_broadcast(rq_b, rq)

            fillers = deque()

            def run_chunk(chunk):
                tiles = {pt: pp1.tile([P, 512], F32, tag="s1", name=f"s1ps{pt}")
                         for pt in chunk}
                for kt in range(16):
                    for pt in chunk:
                        pw = 128 if pt < 16 else 64
                        nc.tensor.matmul(
                            tiles[pt][:pw, :],
                            lhsT=wa_sb[:, kt, pt * 128:pt * 128 + pw],
                            rhs=xt_sb[:, kt, :],
                            start=(kt == 0), stop=(kt == 15))
                    if kt >= 1 and fillers:
                        fillers.popleft()()
                return tiles

            def drain_chunk(chunk, tiles):
                """psum->latt copies (alt Act/DVE) + squares (Pool, from the
                bf16 latt copy so the PSUM bank frees after one read)."""
                sqs = []
                for i, pt in enumerate(chunk):
                    pw = 128 if pt < 16 else 64
                    if i % 2 == 0:
                        nc.scalar.copy(latt[:pw, pt, :], tiles[pt][:pw, :])
                    else:
                        nc.vector.tensor_copy(latt[:pw, pt, :], tiles[pt][:pw, :])
                    if pt < 16:
                        sq_t = sqp.tile([P, 512], BF16, tag="sq")
                        nc.gpsimd.tensor_mul(sq_t, latt[:, pt, :], latt[:, pt, :])
                        sqs.append((pt, sq_t))
                return sqs

            # C1: kv-latent pt 12-15
            tiles = run_chunk(CHUNKS[0])
            sqs = drain_chunk(CHUNKS[0], tiles)
            # C2 fillers: kv ssq + rkv chain (the later V copies wait on
            # rkv_b, whose reciprocal runs on the same DVE queue, so the
            # chain must be emitted before any V copy)
            for pt, sq_t in sqs:
                fillers.append((lambda e=ssq_mm(pt), s=sq_t: e(s)))
            fillers.append(rkv_chain)

            tiles = run_chunk(CHUNKS[1])        # pt 16, 0, 1, 2
            sqs2 = drain_chunk(CHUNKS[1], tiles)

            # k_pe rope (unnormed): latt[:, 16, :] rows [e(32)|o(32)]
            kp = nrm.tile([32, 4, 512], BF16, tag="krope")
            xo_c = nrm.tile([32, 512], BF16, tag="kxo")
            nc.scalar.dma_start(out=xo_c, in_=latt[32:64, 16, :])
            xe = latt[0:32, 16, :]
            c32, s32 = cos_sb[0:32, :], sin_sb[0:32, :]
            nc.vector.tensor_mul(kp[:, 0, :], xe, c32)
            nc.vector.tensor_mul(kp[:, 1, :], xe, s32)
            nc.vector.tensor_mul(kp[:, 2, :], xo_c, s32)
            nc.vector.tensor_mul(kp[:, 3, :], xo_c, c32)
            nc.vector.tensor_sub(kpe_sb[0:32, :], kp[:, 0, :], kp[:, 2, :])
            yi = nrm.tile([32, 512], BF16, tag="kyi")
            nc.vector.tensor_add(yi, kp[:, 1, :], kp[:, 3, :])
            nc.scalar.dma_start(out=kpe_sb[32:64, :], in_=yi)
            for j in range(NC_):
                nc.scalar.dma_start(out=_blk(SEND0A, j, KPE_O, 64), in_=kpe_sb)

            for pt, sq_t in sqs2:
                fillers.append((lambda e=ssq_mm(pt), s=sq_t: e(s)))
            tiles = run_chunk(CHUNKS[2])        # pt 3-6
            sqs = drain_chunk(CHUNKS[2], tiles)
            # V phase between C3 and C4: wkvb has landed mid-stream, and the
            # 13.6us of V matmuls give the trailing wa/wqb DMAs time to land
            for gi in range(2):
                for rt in range(4):
                    v_group(0, gi, rt)
            for pt, sq_t in sqs:
                fillers.append((lambda e=ssq_mm(pt), s=sq_t: e(s)))
            tiles = run_chunk(CHUNKS[3])        # pt 7-10
            sqs = drain_chunk(CHUNKS[3], tiles)
            for pt, sq_t in sqs:
                fillers.append((lambda e=ssq_mm(pt), s=sq_t: e(s)))
            tiles = run_chunk(CHUNKS[4])        # pt 11
            while fillers:
                fillers.popleft()()
            sqs = drain_chunk(CHUNKS[4], tiles)
            for pt, sq_t in sqs:
                ssq_mm(pt)(sq_t)
            rq_chain()

            # A2A-0a: V of even heads + kpe (kpe sends already queued)
            for j in range(NC_):
                nc.scalar.dma_start(
                    out=SEND0A[j, VA_O:VA_O + 4 * 128 * 128].rearrange(
                        "(rt p c) -> p rt c", rt=4, p=128),
                    in_=v_sb[:, :, j * 128:(j + 1) * 128])
            _a2a(SEND0A, RECV0A)

        # ---------------- Stage 2 remainder: K/Q with per-parity A2As ------
        s2out = pctx.enter_context(tc.tile_pool(name=f"s2out{rep}", bufs=1))
        with ExitStack() as sctx:
            pp3 = sctx.enter_context(tc.tile_pool(name=f"ps3{rep}", bufs=4, space="PSUM"))
            rp = sctx.enter_context(tc.tile_pool(name=f"qrope{rep}", bufs=2))
            wqb_b = s2out.tile([P, 12 - NQA, NH * 192], BF16)
            for kt in range(NQA, 12):
                nc.sync.dma_start(out=wqb_b[:, kt - NQA, :], in_=wqb_v[:, kt, :])

            def wqb_t(kt):
                return wqb_a[:, kt, :] if kt < NQA else wqb_b[:, kt - NQA, :]

            kt_sb = s2out.tile([P, 16, 512], BF16)
            qt_sb = s2out.tile([P, 24, 512], BF16)

            def k_par(par):
                for h in range(par, NH, 2):
                    ps = pp3.tile([P, 512], F32)
                    for kt in range(4):
                        nc.tensor.matmul(
                            ps, lhsT=wkvb_sb[:, kt, h * 256:h * 256 + 128],
                            rhs=latt[:, 12 + kt, :], start=(kt == 0), stop=(kt == 3))
                    nc.vector.tensor_mul(kt_sb[:, h, :], ps, rkv_b)

            def q_tiles(tiles_):
                for i, pt in enumerate(tiles_):
                    ps = pp3.tile([P, 512], F32)
                    c0 = _qcol(pt)
                    for kt in range(12):
                        nc.tensor.matmul(
                            ps, lhsT=wqb_t(kt)[:, c0:c0 + 128],
                            rhs=latt[:, kt, :], start=(kt == 0), stop=(kt == 11))
                    nc.vector.tensor_mul(qt_sb[:, pt, :], ps, rq_b)

            def v_par1():
                for gi in range(2):
                    for rt in range(4):
                        v_group(1, gi, rt)

            def send_kqr(par, SEND):
                for j in range(NC_):
                    h = 2 * j + par
                    nc.scalar.dma_start(out=_blk(SEND, j, KN_O, 128),
                                        in_=kt_sb[:, h, :])
                    pe = (h % 4) * 32
                    nc.scalar.dma_start(out=_blk(SEND, j, QR_O, 32),
                                        in_=qt_sb[pe:pe + 32, 16 + h // 4, :])
                    nc.scalar.dma_start(out=_blk(SEND, j, QR_O + 32 * 512, 32),
                                        in_=qt_sb[pe:pe + 32, 20 + h // 4, :])

            def send_qn(par, SEND, off):
                for j in range(NC_):
                    nc.scalar.dma_start(out=_blk(SEND, j, off, 128),
                                        in_=qt_sb[:, 2 * j + par, :])

            def send_v(par, SEND, voff):
                for j in range(NC_):
                    nc.scalar.dma_start(
                        out=SEND[j, voff:voff + 4 * 128 * 128].rearrange(
                            "(rt p c) -> p rt c", rt=4, p=128),
                        in_=v_sb[:, :, par * 1024 + j * 128:par * 1024 + (j + 1) * 128])

            # A2A-0b1: K/Qr of even heads. v_par(1) slots between: it has no
            # wqb dependency, buying the wqb_b stream time to land before the
            # Q matmuls reach kt >= NQA
            k_par(0)
            v_par1()
            q_tiles(range(16, 24))      # all rope tiles
            # Q rope in place (bf16 temps; e-tiles 16+j vs o-tiles 20+j)
            for j in range(4):
                et = qt_sb[:, 16 + j, :]
                ot = qt_sb[:, 20 + j, :]
                t = rp.tile([P, 4, 512], BF16, tag="qr")
                nc.vector.tensor_mul(t[:, 0, :], et, cos_sb)
                nc.vector.tensor_mul(t[:, 1, :], et, sin_sb)
                nc.vector.tensor_mul(t[:, 2, :], ot, sin_sb)
                nc.vector.tensor_mul(t[:, 3, :], ot, cos_sb)
                nc.vector.tensor_sub(et, t[:, 0, :], t[:, 2, :])
                nc.vector.tensor_add(ot, t[:, 1, :], t[:, 3, :])
            send_kqr(0, SEND0B1)
            _a2a(SEND0B1, RECV0B1)

            # A2A-0b2: Qn of even heads; per-dest sends fire as tiles finish
            for j in range(NC_):
                q_tiles([2 * j])
                nc.scalar.dma_start(out=_blk(SEND0B2, j, 0, 128),
                                    in_=qt_sb[:, 2 * j, :])
            _a2a(SEND0B2, RECV0B2)

            # A2A-1: everything for odd heads
            k_par(1)
            q_tiles(range(1, 16, 2))
            send_kqr(1, SEND1)
            send_qn(1, SEND1, QN1_O)
            send_v(1, SEND1, V_O)
            _a2a(SEND1, RECV1)
      # projection pools (latt/wkvb/wqb/qt/kt/v) freed here
      if True:
        # ---------------- Stage 4: attention (hl outer, b inner) -----------
        wop = ctx.enter_context(tc.tile_pool(name=f"wo{rep}", bufs=1))
        wo_sb = wop.tile([P, 16, 2048], BF16)
        wo_v = WO[:, :].rearrange("(kt p) n -> p kt n", p=P)
        for kt in range(16):
            # SWDGE ring: bulk prefetch off the attention recv rings
            nc.gpsimd.dma_start(out=wo_sb[:, kt, :], in_=wo_v[:, kt, :])
        otf = wop.tile([P, 16, 512], BF16)

        with ExitStack() as sctx:
            asm = sctx.enter_context(tc.tile_pool(name=f"asm{rep}", bufs=1))
            ptp = sctx.enter_context(tc.tile_pool(name=f"pt{rep}", bufs=6))
            ppS = sctx.enter_context(tc.tile_pool(name=f"psS{rep}", bufs=3, space="PSUM"))
            ppO = sctx.enter_context(tc.tile_pool(name=f"psO{rep}", bufs=4, space="PSUM"))
            ppD = sctx.enter_context(tc.tile_pool(name=f"psD{rep}", bufs=1, space="PSUM"))
            sml = sctx.enter_context(tc.tile_pool(name=f"sml{rep}", bufs=3))
            otp = sctx.enter_context(tc.tile_pool(name=f"ot{rep}", bufs=2))

            kpool = sctx.enter_context(tc.tile_pool(name=f"kpe{rep}", bufs=1))
            kpe_all = kpool.tile([64, 8, 512], BF16)
            nc.sync.dma_start(
                out=kpe_all,
                in_=RECV0A[:, KPE_O:KPE_O + 64 * 512].rearrange(
                    "s (p c) -> p s c", c=512))

            # hoisted input loads for all (hl, b); hl=1 gates on RECV1
            ain = {}
            for hl in range(2):
                RECV = RECV0B1 if hl == 0 else RECV1
                RECVQ = RECV0B2 if hl == 0 else RECV1
                qoff = 0 if hl == 0 else QN1_O
                RECVV = RECV0A if hl == 0 else RECV1
                voff = VA_O if hl == 0 else V_O
                for b in range(B):
                    ktn = asm.tile([P, 4, 512], BF16, name=f"ktn{hl}{b}")
                    qtn = asm.tile([P, 4, 512], BF16, name=f"qtn{hl}{b}")
                    qtr = asm.tile([64, 4, 512], BF16, name=f"qtr{hl}{b}")
                    vt = asm.tile([P, 16, 128], BF16, name=f"vt{hl}{b}")
                    s0 = 4 * b
                    nc.sync.dma_start(
                        out=ktn,
                        in_=RECV[s0:s0 + 4, KN_O:KN_O + 128 * 512].rearrange(
                            "s (p c) -> p s c", c=512))
                    nc.sync.dma_start(
                        out=qtr,
                        in_=RECV[s0:s0 + 4, QR_O:QR_O + 64 * 512].rearrange(
                            "s (p c) -> p s c", c=512))
                    nc.sync.dma_start(
                        out=qtn,
                        in_=RECVQ[s0:s0 + 4, qoff:qoff + 128 * 512].rearrange(
                            "s (p c) -> p s c", c=512))
                    for i in range(4):
                        nc.sync.dma_start(
                            out=vt[:, 4 * i:4 * i + 4, :],
                            in_=RECVV[s0 + i, voff:voff + 4 * 128 * 128].rearrange(
                                "(rt p c) -> p rt c", rt=4, c=128))
                    ain[(hl, b)] = (ktn, qtn, qtr, vt)

            pending = [None]

            def flush_pending():
                if pending[0] is not None:
                    pending[0]()
                    pending[0] = None

            for hl in range(2):
                ot_sb = otp.tile([P, 4096], BF16, tag="ot")
                for b in range(B):
                    ktn, qtn, qtr, vt = ain[(hl, b)]
                    for qg in range(4):
                        psO = ppO.tile([P, 512], F32)
                        nkt = 4 * qg + 4
                        dacc_a = sml.tile([P, 512], BF16, tag="dacca")
                        prev = None
                        for kt in range(nkt):
                            m = kt - 4 * qg
                            lo = 128 * m if (m > 0 and kt >= 2) else 0
                            cs = slice(lo, 512)
                            psS = ppS.tile([P, 512], F32)
                            nc.tensor.matmul(
                                psS[:, cs],
                                lhsT=ktn[:, kt // 4, (kt % 4) * 128:(kt % 4 + 1) * 128],
                                rhs=qtn[:, qg, cs], start=True, stop=False)
                            nc.tensor.matmul(
                                psS[:, cs],
                                lhsT=kpe_all[:, 4 * b + kt // 4, (kt % 4) * 128:(kt % 4 + 1) * 128],
                                rhs=qtr[:, qg, cs], start=False, stop=True)
                            if kt == 2:
                                # deferred softmax-denominator chain of the
                                # previous qg: PE-emitted here so it never
                                # head-of-line blocks this qg's QK stream
                                flush_pending()
                            if prev is not None:
                                ppt, pcs = prev
                                nc.tensor.matmul(psO[:, pcs], lhsT=vt[:, kt - 1, :],
                                                 rhs=ppt[:, pcs],
                                                 start=(kt == 1), stop=False)
                            pt_t = ptp.tile([P, 512], BF16, tag="pt")
                            nc.scalar.activation(pt_t[:, cs], psS[:, cs],
                                                 mybir.ActivationFunctionType.Exp)
                            if m >= 0:
                                nc.vector.tensor_mul(pt_t[:, cs], pt_t[:, cs],
                                                     masks[:, m, cs])
                            if kt == 0:
                                nc.vector.tensor_copy(dacc_a, pt_t)
                            else:
                                nc.vector.tensor_add(dacc_a[:, cs], dacc_a[:, cs],
                                                     pt_t[:, cs])
                            prev = (pt_t, cs)
                        ppt, pcs = prev
                        nc.tensor.matmul(psO[:, pcs], lhsT=vt[:, nkt - 1, :],
                                         rhs=ppt[:, pcs],
                                         start=(nkt == 1), stop=True)

                        def fin(psO=psO, dacc=dacc_a, b=b, qg=qg, ot_sb=ot_sb):
                            psD = ppD.tile([1, 512], F32)
                            nc.tensor.matmul(psD, lhsT=ones_bf, rhs=dacc,
                                             start=True, stop=True)
                            rcp = sml.tile([1, 512], F32, tag="rcp")
                            nc.vector.reciprocal(rcp, psD)
                            rdb = sml.tile([P, 512], F32, tag="rdb")
                            nc.gpsimd.partition_broadcast(rdb, rcp)
                            nc.vector.tensor_mul(
                                ot_sb[:, b * 2048 + qg * 512:b * 2048 + (qg + 1) * 512],
                                psO, rdb)
                        flush_pending()
                        pending[0] = fin
                # ship this head, overlap with next head's attention
                flush_pending()
                nc.scalar.dma_start(
                    out=SEND2[hl][:, :].rearrange("j (p c) -> p j c", c=512),
                    in_=ot_sb[:, :].rearrange("p (j c) -> p j c", c=512))
                _a2a(SEND2[hl], RECV2[hl])
                nc.sync.dma_start(
                    out=otf[:, hl:16:2, :],
                    in_=RECV2[hl][:, :].rearrange("j (p c) -> p j c", c=512))

        # ---------------- Stage 6: out = O^T.T @ WO, hl-split --------------
        with ExitStack() as sctx:
            pp6 = sctx.enter_context(tc.tile_pool(name=f"ps6{rep}", bufs=4, space="PSUM"))
            outp = sctx.enter_context(tc.tile_pool(name=f"outp{rep}", bufs=2))
            acc = outp.tile([P, 4, 2048], F32, tag="acc")
            for rt in range(4):
                for ng in range(4):
                    ps = pp6.tile([P, 512], F32)
                    for i, h in enumerate(range(0, 16, 2)):
                        nc.tensor.matmul(
                            ps, lhsT=otf[:, h, rt * 128:(rt + 1) * 128],
                            rhs=wo_sb[:, h, ng * 512:(ng + 1) * 512],
                            start=(i == 0), stop=(i == 7))
                    nc.scalar.copy(acc[:, rt, ng * 512:(ng + 1) * 512], ps)
            for rt in range(4):
                out_t = outp.tile([P, 2048], F32, tag="outt")
                for ng in range(4):
                    ps = pp6.tile([P, 512], F32)
                    for i, h in enumerate(range(1, 16, 2)):
                        nc.tensor.matmul(
                            ps, lhsT=otf[:, h, rt * 128:(rt + 1) * 128],
                            rhs=wo_sb[:, h, ng * 512:(ng + 1) * 512],
                            start=(i == 0), stop=(i == 7))
                    nc.vector.tensor_add(
                        out_t[:, ng * 512:(ng + 1) * 512],
                        acc[:, rt, ng * 512:(ng + 1) * 512], ps)
                    nc.sync.dma_start(
                        out=OUT[rt * 128:(rt + 1) * 128, ng * 512:(ng + 1) * 512],
                        in_=out_t[:, ng * 512:(ng + 1) * 512])


# ---------------------------------------------------------------------------
# Host-side prep
# ---------------------------------------------------------------------------

def _bf(a):
    return np.asarray(a, dtype=np.float32).astype(BF16NP)


def _prep_weights(wq_a, q_norm_w, wq_b, wkv_a, kv_norm_w, wkv_b, wo,
                  freqs_cos, freqs_sin):
    wkv_a_lat = wkv_a[:, :KV_LORA]
    wkv_a_rope = wkv_a[:, KV_LORA:]
    wkv_a_rope = np.concatenate([wkv_a_rope[:, 0::2], wkv_a_rope[:, 1::2]], axis=1)
    WAh = np.concatenate([wq_a, wkv_a_lat, wkv_a_rope], axis=1)      # [2048, 2112]

    wqb = (wq_b * SCALE) * q_norm_w[:, None]
    wqb = wqb.reshape(Q_LORA, NH, 192)
    rope_e = wqb[:, :, NOPE + 0::2].reshape(Q_LORA, NH * 32)
    rope_o = wqb[:, :, NOPE + 1::2].reshape(Q_LORA, NH * 32)
    nope = wqb[:, :, :NOPE]                                           # [QL, 16, 128]
    nope_even = nope[:, 0::2, :].reshape(Q_LORA, 8 * NOPE)
    nope_odd = nope[:, 1::2, :].reshape(Q_LORA, 8 * NOPE)
    # column order must match _qcol(): [rope_e | rope_o | even nope | odd]
    WQBh = np.concatenate([rope_e, rope_o, nope_even, nope_odd], axis=1)

    WKVBh = wkv_b * kv_norm_w[:, None]                                # [512, 4096]
    pos = np.arange(R) % S
    COS = freqs_cos[pos].astype(np.float32)                           # [4096, 32]
    SIN = freqs_sin[pos].astype(np.float32)
    p = np.arange(128)[:, None]
    c = np.arange(512)[None, :]
    MASK = np.stack([(c - 128 * m - p >= 0) for m in range(4)],
                    axis=1).astype(np.float32)                        # [128, 4, 512]
    return dict(WA=_bf(WAh), WQB=_bf(WQBh), WKVB=_bf(WKVBh), WO=_bf(wo),
                COS=COS, SIN=SIN, MASK=_bf(MASK.reshape(128, 2048)))


def _prep_in_maps(inputs):
    x = np.asarray(inputs["x"], dtype=np.float32).reshape(R, 2048)
    W = _prep_weights(
        np.asarray(inputs["wq_a"]), np.asarray(inputs["q_norm_w"]),
        np.asarray(inputs["wq_b"]), np.asarray(inputs["wkv_a"]),
        np.asarray(inputs["kv_norm_w"]), np.asarray(inputs["wkv_b"]),
        np.asarray(inputs["wo"]),
        np.asarray(inputs["freqs_cos"]), np.asarray(inputs["freqs_sin"]))
    in_maps = []
    for c in range(NC_):
        rows = slice(c * LR, (c + 1) * LR)
        in_maps.append({
            "xt": np.ascontiguousarray(x[rows].T).astype(BF16NP),
            "wa": W["WA"], "wqb": W["WQB"], "wkvb": W["WKVB"], "wo": W["WO"],
            "cost": np.ascontiguousarray(np.tile(W["COS"][rows].T, (4, 1))),
            "sint": np.ascontiguousarray(np.tile(W["SIN"][rows].T, (4, 1))),
            "masks": W["MASK"],
        })
    return in_maps


prep_in_maps = _prep_in_maps

_NC_CACHE = []


def _get_nc():
    if not _NC_CACHE:
        _NC_CACHE.append(build_kernel())
    return _NC_CACHE[0]


def kernel(**inputs) -> np.ndarray:
    in_maps = _prep_in_maps(inputs)
    nc = _get_nc()
    res = run_bass_kernel_spmd(nc, in_maps, core_ids=list(range(NC_)))
    outs = [res.results[c]["out"] for c in range(NC_)]
    return np.concatenate(outs, axis=0).reshape(B, S, 2048).astype(np.float32)
